# revision 6
# baseline (speedup 1.0000x reference)
"""Pairwise L2 distance kernel: x [4096,768], anchors [100,64,768] -> [4096,100,64].

Distributed over 8 TRN2 NeuronCores as a 2x4 grid: batch (4096) split in 2,
anchor index (6400) split in 4.  Each core computes a [2048,1600] output block
as sqrt(x2[b] + a2[j] - 2*x@A^T).

The x@A^T matmul runs in fp8e4m3 with DoubleRow (K=256 per pass, fp32 PSUM
accumulate).  Row norms x2 come from ACT Square+accumulate over an fp8 copy of
x; anchor norms a2 from fp8 squares of at (split across DVE/GPSIMD/ACT) summed
and broadcast by a DoubleRow ones-matmul.  The epilogue keeps everything bf16
(2x ACT throughput for the Sqrt): t = psum + (-0.5*a2) on DVE, out =
sqrt(-2*t + x2) on ACT.  Inputs arrive as k-pair (at) / m-tile (xt, xo) DMA
slices issued from the GPSIMD queue so the PE stream starts early.  Host does
layout transforms only (transpose, dtype cast, partition packing).
"""

import sys

import numpy as np

for _p in ("/opt/trn_rl_repo", "/root/.axon_site/_ro/trn_rl_repo"):
    if _p not in sys.path:
        sys.path.append(_p)

import ml_dtypes

import concourse.bass as bass
import concourse.tile as tile
from concourse import bacc, mybir
from concourse.bass import ts
from concourse.bass_utils import run_bass_kernel_spmd

B, C, A, E = 4096, 100, 64, 768
J = C * A                 # 6400 flattened anchors
RB, RJ = 2, 4             # batch groups x anchor groups = 8 cores
MB = B // RB              # 2048 batch rows per core
NJ = J // RJ              # 1600 anchor cols per core
KT = E // 128             # 6 contraction tiles of 128
K2 = KT // 2              # 3 DoubleRow k-pair passes
MT = MB // 128            # 16 m-tiles per core
XT_Q = 8                  # xt arrives in 8 DMA slices (2 m-tiles each)
XO_Q = 4                  # xo arrives in 4 DMA slices (4 m-tiles each)
N_CHUNKS = [(0, 512), (512, 512), (1024, 512), (1536, 64)]
PSW = 2048                # psum tile width (4 banks)
N_WARM = 18               # dummy matmuls spanning the input-DMA head

FP8 = mybir.dt.float8e4
BF16 = mybir.dt.bfloat16
F32 = mybir.dt.float32
NP_FP8 = ml_dtypes.float8_e4m3
NP_BF16 = ml_dtypes.bfloat16


def pack_rows(a2d: np.ndarray) -> np.ndarray:
    """[n*128, F] -> [128, n*F]: row r=k*128+p lands at partition p, block k.
    Makes each SBUF partition's data contiguous in DRAM."""
    n = a2d.shape[0] // 128
    return np.ascontiguousarray(
        a2d.reshape(n, 128, a2d.shape[1]).transpose(1, 0, 2).reshape(128, -1)
    )


def pack_xt(xtg: np.ndarray) -> np.ndarray:
    """[E, MB] -> [128, MT*KT*128] m-major: partition p holds, for each m-tile,
    that tile's KT k-blocks contiguously, so a per-m DMA slice is one fat
    descriptor per partition."""
    return np.ascontiguousarray(
        xtg.reshape(KT, 128, MT, 128).transpose(1, 2, 0, 3).reshape(128, -1)
    )


def build_graph() -> bass.Bass:
    mt_q = MT // XO_Q
    nc = bacc.Bacc(None, target_bir_lowering=False, debug=False, num_devices=8)
    at_ext = nc.declare_dram_parameter("at", [128, KT * NJ], FP8, isOutput=False)
    xt_ext = nc.declare_dram_parameter("xt", [128, MT * KT * 128], FP8, isOutput=False)
    xo_ext = nc.declare_dram_parameter("xo", [128, MT * E], FP8, isOutput=False)
    out_ext = nc.declare_dram_parameter("out", [MB, NJ], BF16, isOutput=True)

    with tile.TileContext(nc) as tc:
        with (
            tc.tile_pool(name="big", bufs=1) as big,
            tc.tile_pool(name="atp", bufs=K2) as atp,
            tc.tile_pool(name="sqp", bufs=K2) as sqp,
            tc.tile_pool(name="xtp", bufs=XT_Q) as xtp,
            tc.tile_pool(name="xop", bufs=XO_Q) as xop,
            tc.tile_pool(name="work", bufs=4) as work,
            tc.tile_pool(name="outs", bufs=4) as outs,
            tc.tile_pool(name="psum", bufs=2, space=bass.MemorySpace.PSUM) as psp,
        ):
            # --- constants (DVE memsets at t~0)
            warm_lhs = big.tile([128, 64], BF16, tag="wl")
            nc.vector.memset(warm_lhs, 1.0)
            neg2 = big.tile([128, 2, 128], FP8, tag="n2")   # DoubleRow -0.5 lhsT
            nc.vector.memset(neg2, -0.5)
            warm_src = big.tile([128, 512], BF16, tag="ws")
            nc.vector.memset(warm_src, 0.125)
            dummy = big.tile([128, 1], BF16, tag="dm")

            # --- input DMAs issued from GPSIMD (cheap config) at t~0,
            # roughly in priority order.
            at_r = at_ext[:].rearrange("p (q r n) -> p q r n", q=K2, r=2)
            at_s, sq_s = [], []
            for q in range(K2):
                a_t = atp.tile([128, 2, NJ], FP8, tag="at", name=f"at{q}")
                nc.gpsimd.dma_start(out=a_t, in_=at_r[:, q, :, :])
                at_s.append(a_t)
                sq_s.append(sqp.tile([128, 2, NJ], FP8, tag="sq", name=f"sq{q}"))
            xt_r = xt_ext[:].rearrange("p (s m k c) -> p s m k c", s=XT_Q, m=2, k=KT)
            xo_r = xo_ext[:].rearrange("p (u e) -> p u e", u=XO_Q)
            xt_s, xo_s = [], []
            for s in range(XT_Q):
                x_t = xtp.tile([128, 2, KT, 128], FP8, tag="xt", name=f"xt{s}")
                nc.gpsimd.dma_start(out=x_t, in_=xt_r[:, s, :, :, :])
                xt_s.append(x_t)
                if s % 2 == 0:
                    u = s // 2
                    o_t = xop.tile([128, mt_q * E], FP8, tag="xo", name=f"xo{u}")
                    nc.gpsimd.dma_start(out=o_t, in_=xo_r[:, u, :])
                    xo_s.append(o_t)

            def xt_sl(m, q):  # lhsT [128, 2, 128] for tile m, k-pair q
                return xt_s[m // 2][:, m % 2, 2 * q : 2 * q + 2, :]

            # --- ACT: load the sqrt table at t~0 (set also contains Square)
            nc.scalar.activation(dummy, warm_src[:, 0:1],
                                 mybir.ActivationFunctionType.Sqrt)

            # --- sq_at = at*at in fp8, one slice per engine as at lands
            nc.vector.tensor_mul(sq_s[0], at_s[0], at_s[0])
            nc.gpsimd.tensor_mul(sq_s[1], at_s[1], at_s[1])
            nc.scalar.activation(sq_s[2], at_s[2],
                                 mybir.ActivationFunctionType.Square)

            # --- x2 per m-tile: ACT Square with accumulate over xo (fp8).
            # Emit the first few eagerly; the rest interleave with the sqrts.
            x2s, sqx = [], []

            def emit_x2(m):
                sq_x = work.tile([128, E], BF16, tag="sqx", name=f"sqx{m}", bufs=2)
                x2 = work.tile([128, 1], F32, tag="x2", name=f"x2_{m}", bufs=MT)
                nc.scalar.activation(
                    sq_x, xo_s[m // mt_q][:, (m % mt_q) * E : (m % mt_q + 1) * E],
                    mybir.ActivationFunctionType.Square, accum_out=x2,
                )
                x2s.append(x2)

            for m in range(4):
                emit_x2(m)

            # --- PE warm-up across the DMA head (p-state ramp)
            warm_ps = psp.tile([128, PSW], F32, tag="ps", name="warm_ps")
            for wi in range(N_WARM):
                nc.tensor.matmul(
                    warm_ps[:64, :512], warm_lhs, warm_src,
                    start=(wi == 0), stop=(wi == N_WARM - 1),
                )

            a2b = big.tile([128, NJ], BF16, tag="a2b")   # -0.5*a2[j] broadcast

            def emit_mains(pts, m):
                for q in range(K2):
                    lhsT = xt_sl(m, q)
                    for n0, w in N_CHUNKS:
                        nc.tensor.matmul(
                            pts[:, n0 : n0 + w], lhsT,
                            at_s[q][:, :, n0 : n0 + w],
                            start=(q == 0), stop=(q == K2 - 1),
                            perf_mode=mybir.MatmulPerfMode.DoubleRow,
                        )

            def emit_a2_setup():
                ps = psp.tile([128, PSW], F32, tag="ps", name="psa2")
                for q in range(K2):
                    for n0, w in N_CHUNKS:
                        nc.tensor.matmul(
                            ps[:, n0 : n0 + w], neg2, sq_s[q][:, :, n0 : n0 + w],
                            start=(q == 0), stop=(q == K2 - 1),
                            perf_mode=mybir.MatmulPerfMode.DoubleRow,
                        )
                nc.vector.tensor_copy(a2b, ps[:, :NJ])

            for m in range(MT):
                pts = psp.tile([128, PSW], F32, tag="ps", name=f"ps{m}")
                emit_mains(pts, m)

                if m == 0:
                    # m0: free the psum slot early (copy), add a2b in place
                    # once it lands; meanwhile PE runs the a2 ones-matmul.
                    t0 = work.tile([128, NJ], BF16, tag="t", name="t0", bufs=2)
                    nc.vector.tensor_copy(t0, pts[:, :NJ])
                    emit_a2_setup()
                    nc.vector.tensor_add(t0, t0, a2b)
                    out_t = outs.tile([128, NJ], BF16, tag="out", name="out0")
                    nc.scalar.activation(
                        out_t, t0, mybir.ActivationFunctionType.Sqrt,
                        bias=x2s[0], scale=-2.0,
                    )
                    nc.sync.dma_start(out=out_ext[ts(0, 128), :], in_=out_t)
                    emit_x2(4)
                    continue

                out_t = outs.tile([128, NJ], BF16, tag="out", name=f"out{m}")
                halves = [(0, NJ)] if m < MT - 2 else [(0, NJ // 2), (NJ // 2, NJ)]
                for h0, h1 in halves:
                    t = work.tile([128, NJ], BF16, tag="t", name=f"t{m}_{h0}",
                                  bufs=2)
                    nc.vector.tensor_add(
                        t[:, : h1 - h0], pts[:, h0:h1], a2b[:, h0:h1]
                    )
                    nc.scalar.activation(
                        out_t[:, h0:h1], t[:, : h1 - h0],
                        mybir.ActivationFunctionType.Sqrt,
                        bias=x2s[m], scale=-2.0,
                    )
                    nc.sync.dma_start(
                        out=out_ext[ts(m, 128), h0:h1], in_=out_t[:, h0:h1]
                    )
                if m + 4 < MT:
                    emit_x2(m + 4)

    nc.compile()
    return nc


def make_in_maps(x32: np.ndarray, a32: np.ndarray) -> list[dict[str, np.ndarray]]:
    xt_f8 = x32.T.astype(NP_FP8)           # [E, B]
    xo_f8 = x32.astype(NP_FP8)             # [B, E]
    at_f8 = a32.T.astype(NP_FP8)           # [E, J]
    in_maps = []
    for c in range(8):
        g, h = c // RJ, c % RJ
        in_maps.append({
            "at": pack_rows(at_f8[:, h * NJ : (h + 1) * NJ]),
            "xt": pack_xt(xt_f8[:, g * MB : (g + 1) * MB]),
            "xo": pack_rows(xo_f8[g * MB : (g + 1) * MB, :]),
        })
    return in_maps


def kernel(x: np.ndarray, anchors: np.ndarray) -> np.ndarray:
    x32 = np.asarray(x, dtype=np.float32)
    a32 = np.asarray(anchors, dtype=np.float32).reshape(J, E)

    nc = build_graph()
    in_maps = make_in_maps(x32, a32)
    results = run_bass_kernel_spmd(nc, in_maps, core_ids=list(range(8))).results

    out = np.empty((B, J), dtype=np.float32)
    for c in range(8):
        g, h = c // RJ, c % RJ
        out[g * MB : (g + 1) * MB, h * NJ : (h + 1) * NJ] = results[c][
            "out"
        ].astype(np.float32)
    return out.reshape(B, C, A)
